# revision 1
# baseline (speedup 1.0000x reference)
"""Trainium2 Bass kernel: depthwise (per-sample, per-channel) 15x15 'same'
true convolution of 1024x3 images of 128x128, data-parallel over 8 NeuronCores.

Formulation (per (bn,c) pair, P=128, K=15, pad=7):
    out[y,x] = sum_{dy,dx} Xpad[y+dy, x+dx] * W[dy,dx],   W = flip(kernel)
y is split into 2 blocks of 64 rows. For block y0 and each dx the contribution
is a matmul with a banded block-Toeplitz stationary operand
    T[i, dx, j] = W[i-j, dx]  (i in 0..77, j in 0..63, band 0 <= i-j < 15)
    out[y0+j, x] += sum_i T[i, dx, j] * Xpad[y0+i, x+dx]
accumulated over the 15 dx values in PSUM. The two y-blocks write PSUM
partitions 0..63 / 64..127 (128x64 column tiling of the PE array). Operands
are fp16 (PSUM accumulates fp32); T and the padded images are prepared
host-side. Sharding: pure data parallel over BN (128 samples x 3 channels
= 384 independent images per core).
"""
import sys

sys.path.insert(0, "/opt/trn_rl_repo")

import numpy as np

_N_CORES = 8
_BN, _C, _P, _K = 1024, 3, 128, 15
_PAIRS_PER_CORE = (_BN // _N_CORES) * _C  # 384

_nc_cache = {}


def _build_nc(n_pairs: int, bufs: int = 6, psum_bufs: int = 4):
    import concourse.bacc as bacc
    import concourse.mybir as mybir
    from concourse import tile

    FP16 = mybir.dt.float16
    FP32 = mybir.dt.float32

    nc = bacc.Bacc("TRN2", target_bir_lowering=False, debug=False)
    xpad_d = nc.dram_tensor("xpad", [n_pairs, 142, 142], FP16, kind="ExternalInput")
    toep_d = nc.dram_tensor("toep", [n_pairs, 78, 15, 64], FP16, kind="ExternalInput")
    out_d = nc.dram_tensor("out", [n_pairs, 128, 128], FP32, kind="ExternalOutput")

    with tile.TileContext(nc) as tc:
        with (
            tc.tile_pool(name="xa", bufs=bufs) as xa_pool,
            tc.tile_pool(name="xb", bufs=bufs) as xb_pool,
            tc.tile_pool(name="tt", bufs=bufs) as tt_pool,
            tc.tile_pool(name="ot", bufs=bufs) as ot_pool,
            tc.tile_pool(name="ps", bufs=psum_bufs, space="PSUM") as ps_pool,
        ):
            for p in range(n_pairs):
                xa = xa_pool.tile([78, 142], FP16, tag="xa")
                xb = xb_pool.tile([78, 142], FP16, tag="xb")
                tt = tt_pool.tile([78, 15, 64], FP16, tag="tt")
                nc.sync.dma_start(out=xa[:], in_=xpad_d[p, 0:78, :])
                nc.sync.dma_start(out=xb[:], in_=xpad_d[p, 64:142, :])
                nc.sync.dma_start(out=tt[:], in_=toep_d[p])

                ps0 = ps_pool.tile([128, 128], FP32, tag="ps0")
                ps1 = ps_pool.tile([128, 128], FP32, tag="ps1")
                for dx in range(15):
                    lhsT = tt[:, dx, :]
                    nc.tensor.matmul(
                        ps0[0:64, :], lhsT, xa[:, dx:dx + 128],
                        start=(dx == 0), stop=(dx == 14),
                    )
                    nc.tensor.matmul(
                        ps1[64:128, :], lhsT, xb[:, dx:dx + 128],
                        start=(dx == 0), stop=(dx == 14),
                    )

                ot = ot_pool.tile([128, 128], FP32, tag="ot")
                nc.vector.tensor_copy(ot[0:64, :], ps0[0:64, :])
                nc.scalar.copy(ot[64:128, :], ps1[64:128, :])
                nc.sync.dma_start(out=out_d[p], in_=ot[:])

    nc.compile()
    return nc


def _host_prep(patches_pairs: np.ndarray, kernels_pairs: np.ndarray):
    """[NP,128,128] f32, [NP,15,15] f32 -> xpad [NP,142,142] fp16,
    toep [NP,78,15,64] fp16 with toep[p,i,dx,j] = flip(kern)[i-j, dx]."""
    NP = patches_pairs.shape[0]
    Xp = np.zeros((NP, 142, 142), dtype=np.float16)
    Xp[:, 7:135, 7:135] = patches_pairs.astype(np.float16)
    W = kernels_pairs[:, ::-1, ::-1].astype(np.float16)
    H = np.zeros((NP, 141, 15), dtype=np.float16)
    H[:, 63:78, :] = W
    s0, s1, s2 = H.strides
    A = np.lib.stride_tricks.as_strided(
        H[:, 63:, :], shape=(NP, 78, 64, 15), strides=(s0, s1, -s1, s2)
    )
    T = np.ascontiguousarray(A.transpose(0, 1, 3, 2))
    return Xp, T


def kernel(patches, kernels, kernel_size, patch_size, fft_size, _collect_results=None):
    """Full inputs in, full output out. Shards BN across 8 cores."""
    from concourse.bass_utils import run_bass_kernel_spmd

    patches = np.asarray(patches)
    kernels = np.asarray(kernels)
    assert patches.shape == (_BN, _C, _P, _P), patches.shape
    assert kernels.shape == (_BN, _C, _K, _K), kernels.shape

    if "nc" not in _nc_cache:
        _nc_cache["nc"] = _build_nc(_PAIRS_PER_CORE)
    nc = _nc_cache["nc"]

    bn_per_core = _BN // _N_CORES
    in_maps = []
    for core in range(_N_CORES):
        sl = slice(core * bn_per_core, (core + 1) * bn_per_core)
        pp = patches[sl].reshape(-1, _P, _P)
        kp = kernels[sl].reshape(-1, _K, _K)
        xpad, toep = _host_prep(pp, kp)
        in_maps.append({"xpad": xpad, "toep": toep})

    res = run_bass_kernel_spmd(nc, in_maps, core_ids=list(range(_N_CORES)))
    if _collect_results is not None:
        _collect_results.append(res)

    out = np.empty((_BN, _C, _P, _P), dtype=np.float32)
    for core in range(_N_CORES):
        sl = slice(core * bn_per_core, (core + 1) * bn_per_core)
        out[sl] = res.results[core]["out"].reshape(bn_per_core, _C, _P, _P)
    return out



# revision 3
# speedup vs baseline: 1.0556x; 1.0556x over previous
"""Trainium2 Bass kernel: depthwise (per-sample, per-channel) 15x15 'same'
true convolution of 1024x3 images of 128x128, data-parallel over 8 NeuronCores.

Formulation (per (bn,c) pair, P=128, K=15, pad=7):
    out[y,x] = sum_{dy,dx} Xp[y+dy, x+dx] * Wf[dy,dx],   Wf = flip(kernel),
    Xp = zero-padded image [142, 143].
Output rows are split into 4 blocks of 32 (j in 0..31). Each block runs on its
own 32-wide column strip of the PE array (tile_position=(0, 32b)) so the four
blocks' matmuls execute concurrently. Contraction (92) packs two dx taps:
segment s in {0,1} holds image rows 32b..32b+45 shifted s columns. Pass t
(t=0..7) covers dx = 2t+s via a moving-operand column offset of 2t;
the stationary Toeplitz slab T[46s+i, t, j] = Wf[i-j, 2t+s] accumulates all 8
passes into PSUM [32, 128] per block.

Data staging (per group of G=8 pairs): images are stored pair-interleaved in
DRAM ([row][pair][143]) so one DMA with 2288-byte runs fills segment 0 of the
x4 tile ([92, 4*G*143]); segment 1 (shift-by-one-column copy) is an
SBUF->SBUF DMA with one descriptor per partition. Toeplitz slabs and fp16
outputs are similarly group-batched. Sharding: pure data parallel over BN
(128 samples x 3 channels = 384 pairs per core).
"""
import sys

sys.path.insert(0, "/opt/trn_rl_repo")

import numpy as np

_N_CORES = 8
_BN, _C, _P, _K = 1024, 3, 128, 15
_PAIRS_PER_CORE = (_BN // _N_CORES) * _C  # 384
_G = 8                       # pairs per DMA group
_NG = _PAIRS_PER_CORE // _G  # 48
_XW = 143                    # padded image width (cols 0..142)
_XH = 142                    # padded image height
_ROWP = _G * _XW             # elems per padded row across a group (1144)
_GRP = _XH * _ROWP           # elems per group image block (162448)
_SLAB = 4 * _ROWP            # x4 tile free elems (4576) + 1 slack

_nc_cache = {}


def _build_nc(bufs: int = 2, psum_bufs: int = 4):
    import concourse.bacc as bacc
    import concourse.mybir as mybir
    from concourse import bass, tile

    FP16 = mybir.dt.float16
    FP32 = mybir.dt.float32

    nc = bacc.Bacc("TRN2", target_bir_lowering=False, debug=False)
    xpad_d = nc.dram_tensor("xpad", [_NG * _GRP + 64], FP16, kind="ExternalInput")
    toep_d = nc.dram_tensor("toep", [_NG, 92, _G * 8 * 32], FP16, kind="ExternalInput")
    out_d = nc.dram_tensor("out", [_NG, 128, _G * 128], FP16, kind="ExternalOutput")

    with tile.TileContext(nc) as tc:
        with (
            tc.tile_pool(name="x4", bufs=bufs) as x4_pool,
            tc.tile_pool(name="tt", bufs=bufs) as tt_pool,
            tc.tile_pool(name="ot", bufs=bufs) as ot_pool,
            tc.tile_pool(name="ps", bufs=psum_bufs, space="PSUM") as ps_pool,
        ):
            for grp in range(_NG):
                x4 = x4_pool.tile([92, _SLAB + 1], FP16, tag="x4")
                tt = tt_pool.tile([92, _G * 8 * 32], FP16, tag="tt")
                ot = ot_pool.tile([128, _G * 128], FP16, tag="ot")

                # seg0: partitions 0..45 <- rows 32b+i of each pair (interleaved)
                src0 = bass.AP(
                    tensor=xpad_d.tensor if hasattr(xpad_d, "tensor") else xpad_d,
                    offset=grp * _GRP,
                    ap=[[_ROWP, 46], [32 * _ROWP, 4], [_XW, _G], [1, _XW]],
                )
                nc.sync.dma_start(out=x4[0:46, 0:_SLAB], in_=src0)
                # seg1: partitions 46..91 <- same, shifted one column
                nc.sync.dma_start(out=x4[46:92, 0:_SLAB], in_=x4[0:46, 1:_SLAB + 1])
                # Toeplitz slabs
                nc.sync.dma_start(out=tt[:], in_=toep_d[grp])

                for g in range(_G):
                    ps = ps_pool.tile([128, 128], FP32, tag="ps")
                    for t in range(8):
                        for b in range(4):
                            nc.tensor.matmul(
                                ps[32 * b:32 * b + 32, :],
                                tt[0:92, (g * 8 + t) * 32:(g * 8 + t) * 32 + 32],
                                x4[0:92, (b * _G + g) * _XW + 2 * t:
                                   (b * _G + g) * _XW + 2 * t + 128],
                                start=(t == 0), stop=(t == 7),
                                tile_position=(0, 32 * b),
                            )
                    nc.vector.tensor_copy(
                        ot[0:64, g * 128:(g + 1) * 128], ps[0:64, :])
                    nc.scalar.copy(
                        ot[64:128, g * 128:(g + 1) * 128], ps[64:128, :])

                nc.sync.dma_start(out=out_d[grp], in_=ot[:])

    nc.compile()
    return nc


def _host_prep(patches_pairs: np.ndarray, kernels_pairs: np.ndarray):
    """[NP,128,128] f32, [NP,15,15] f32 -> (xpad flat fp16, toep fp16).

    xpad: [NG*142*G*143 + 64] with layout [grp][row 142][pair G][col 143],
    zero-padded images at rows/cols 7..134.
    toep: [NG, 92, G*8*32] with T[p][46s+i, t, j] = Wf[i-j, 2t+s]
    (0 <= i-j < 15, dx = 2t+s <= 14), layout [grp][i_stack][pair][t][j].
    """
    NP = patches_pairs.shape[0]
    assert NP == _PAIRS_PER_CORE
    Xp = np.zeros((_NG, _G, _XH, _XW), dtype=np.float16)
    Xp[:, :, 7:135, 7:135] = patches_pairs.reshape(_NG, _G, 128, 128)
    xpad = np.zeros(_NG * _GRP + 64, dtype=np.float16)
    xpad[:_NG * _GRP] = np.ascontiguousarray(
        Xp.transpose(0, 2, 1, 3)).reshape(-1)

    Wf = np.ascontiguousarray(
        kernels_pairs[:, ::-1, ::-1]).astype(np.float16)  # [NP, 15, 15]
    T = np.zeros((NP, 2, 46, 8, 32), dtype=np.float16)
    j = np.arange(32)
    for dy in range(15):
        for t in range(8):
            for s in range(2):
                dx = 2 * t + s
                if dx > 14:
                    continue
                T[:, s, j + dy, t, j] = Wf[:, dy, dx][:, None]
    T = T.reshape(_NG, _G, 92, 8 * 32).transpose(0, 2, 1, 3)
    toep = np.ascontiguousarray(T).reshape(_NG, 92, _G * 8 * 32)
    return xpad, toep


def kernel(patches, kernels, kernel_size, patch_size, fft_size, _collect_results=None):
    """Full inputs in, full output out. Shards BN across 8 cores."""
    from concourse.bass_utils import run_bass_kernel_spmd

    patches = np.asarray(patches)
    kernels = np.asarray(kernels)
    assert patches.shape == (_BN, _C, _P, _P), patches.shape
    assert kernels.shape == (_BN, _C, _K, _K), kernels.shape

    if "nc" not in _nc_cache:
        _nc_cache["nc"] = _build_nc()
    nc = _nc_cache["nc"]

    bn_per_core = _BN // _N_CORES
    in_maps = []
    for core in range(_N_CORES):
        sl = slice(core * bn_per_core, (core + 1) * bn_per_core)
        pp = patches[sl].reshape(-1, _P, _P)
        kp = kernels[sl].reshape(-1, _K, _K)
        xpad, toep = _host_prep(pp, kp)
        in_maps.append({"xpad": xpad, "toep": toep})

    res = run_bass_kernel_spmd(nc, in_maps, core_ids=list(range(_N_CORES)))
    if _collect_results is not None:
        _collect_results.append(res)

    out = np.empty((_BN, _C, _P, _P), dtype=np.float32)
    for core in range(_N_CORES):
        sl = slice(core * bn_per_core, (core + 1) * bn_per_core)
        o = res.results[core]["out"].reshape(_NG, 128, _G, 128)
        out[sl] = o.transpose(0, 2, 1, 3).reshape(
            bn_per_core, _C, _P, _P).astype(np.float32)
    return out


# revision 5
# speedup vs baseline: 1.1507x; 1.0901x over previous
"""Trainium2 Bass kernel: depthwise (per-sample, per-channel) 15x15 'same'
true convolution of 1024x3 images of 128x128, data-parallel over 8 NeuronCores.

Formulation (per (bn,c) pair, P=128, K=15, pad=7):
    out[y,x] = sum_{dy,dx} Xp[y+dy, x+dx] * Wf[dy,dx],   Wf = flip(kernel),
    Xp = zero-padded image [142, 143].
Output rows are split into 4 blocks of 32 (j in 0..31). Each block runs on its
own 32-wide column strip of the PE array (tile_position=(0, 32b)) so the four
blocks' matmuls execute concurrently. Contraction (92) packs two dx taps:
segment s in {0,1} holds image rows 32b..32b+45 shifted s columns. Pass t
(t=0..7) covers dx = 2t+s via a moving-operand column offset of 2t;
the stationary Toeplitz slab T[46s+i, t, j] = Wf[i-j, 2t+s] accumulates all 8
passes into PSUM [32, 128] per block.

Data staging (per group of G=8 pairs): images are stored pair-interleaved in
DRAM ([row][pair][143]) so one DMA with 2288-byte runs fills segment 0 of the
x4 tile ([92, 4*G*143]); segment 1 (shift-by-one-column copy) is an
SBUF->SBUF DMA with one descriptor per partition. Toeplitz slabs and fp16
outputs are similarly group-batched. Sharding: pure data parallel over BN
(128 samples x 3 channels = 384 pairs per core).
"""
import sys

sys.path.insert(0, "/opt/trn_rl_repo")

import numpy as np

_N_CORES = 8
_BN, _C, _P, _K = 1024, 3, 128, 15
_PAIRS_PER_CORE = (_BN // _N_CORES) * _C  # 384
_G = 8                       # pairs per DMA group
_NG = _PAIRS_PER_CORE // _G  # 48
_XW = 143                    # padded image width (cols 0..142)
_XH = 142                    # padded image height
_ROWP = _G * _XW             # elems per padded row across a group (1144)
_GRP = _XH * _ROWP           # elems per group image block (162448)
_SLAB = 4 * _ROWP            # x4 tile free elems (4576) + 1 slack

_nc_cache = {}


def _build_nc(bufs: int = 2, psum_bufs: int = 4):
    import concourse.bacc as bacc
    import concourse.mybir as mybir
    from concourse import bass, tile

    FP16 = mybir.dt.float16
    FP32 = mybir.dt.float32

    nc = bacc.Bacc("TRN2", target_bir_lowering=False, debug=False)
    xpad_d = nc.dram_tensor("xpad", [_NG * _GRP + 64], FP16, kind="ExternalInput")
    toep_d = nc.dram_tensor("toep", [_NG, 92, _G * 8 * 32], FP16, kind="ExternalInput")
    out_d = nc.dram_tensor("out", [_NG, 128, _G * 128], FP16, kind="ExternalOutput")

    with tile.TileContext(nc) as tc:
        with (
            tc.tile_pool(name="x4", bufs=bufs) as x4_pool,
            tc.tile_pool(name="tt", bufs=bufs) as tt_pool,
            tc.tile_pool(name="ot", bufs=bufs) as ot_pool,
            tc.tile_pool(name="ps", bufs=psum_bufs, space="PSUM") as ps_pool,
        ):
            for grp in range(_NG):
                x4 = x4_pool.tile([92, _SLAB + 1], FP16, tag="x4")
                tt = tt_pool.tile([92, _G * 8 * 32], FP16, tag="tt")
                ot = ot_pool.tile([128, _G * 128], FP16, tag="ot")

                # seg0: partitions 0..45 <- rows 32b+i of each pair (interleaved)
                src0 = bass.AP(
                    tensor=xpad_d.tensor if hasattr(xpad_d, "tensor") else xpad_d,
                    offset=grp * _GRP,
                    ap=[[_ROWP, 46], [32 * _ROWP, 4], [_XW, _G], [1, _XW]],
                )
                nc.sync.dma_start(out=x4[0:46, 0:_SLAB], in_=src0)
                # seg1: partitions 46..91 <- same, shifted one column
                nc.scalar.dma_start(out=x4[46:92, 0:_SLAB], in_=x4[0:46, 1:_SLAB + 1])
                # Toeplitz slabs
                nc.gpsimd.dma_start(out=tt[:], in_=toep_d[grp])

                for g in range(_G):
                    ps = ps_pool.tile([128, 128], FP32, tag="ps")
                    for t in range(8):
                        for b in range(4):
                            nc.tensor.matmul(
                                ps[32 * b:32 * b + 32, :],
                                tt[0:92, (g * 8 + t) * 32:(g * 8 + t) * 32 + 32],
                                x4[0:92, (b * _G + g) * _XW + 2 * t:
                                   (b * _G + g) * _XW + 2 * t + 128],
                                start=(t == 0), stop=(t == 7),
                                tile_position=(0, 32 * b),
                            )
                    nc.vector.tensor_copy(
                        ot[0:64, g * 128:(g + 1) * 128], ps[0:64, :])
                    nc.scalar.copy(
                        ot[64:128, g * 128:(g + 1) * 128], ps[64:128, :])

                nc.scalar.dma_start(out=out_d[grp], in_=ot[:])

    nc.compile()
    return nc


def _host_prep(patches_pairs: np.ndarray, kernels_pairs: np.ndarray):
    """[NP,128,128] f32, [NP,15,15] f32 -> (xpad flat fp16, toep fp16).

    xpad: [NG*142*G*143 + 64] with layout [grp][row 142][pair G][col 143],
    zero-padded images at rows/cols 7..134.
    toep: [NG, 92, G*8*32] with T[p][46s+i, t, j] = Wf[i-j, 2t+s]
    (0 <= i-j < 15, dx = 2t+s <= 14), layout [grp][i_stack][pair][t][j].
    """
    NP = patches_pairs.shape[0]
    assert NP == _PAIRS_PER_CORE
    Xp = np.zeros((_NG, _G, _XH, _XW), dtype=np.float16)
    Xp[:, :, 7:135, 7:135] = patches_pairs.reshape(_NG, _G, 128, 128)
    xpad = np.zeros(_NG * _GRP + 64, dtype=np.float16)
    xpad[:_NG * _GRP] = np.ascontiguousarray(
        Xp.transpose(0, 2, 1, 3)).reshape(-1)

    Wf = np.ascontiguousarray(
        kernels_pairs[:, ::-1, ::-1]).astype(np.float16)  # [NP, 15, 15]
    T = np.zeros((NP, 2, 46, 8, 32), dtype=np.float16)
    j = np.arange(32)
    for dy in range(15):
        for t in range(8):
            for s in range(2):
                dx = 2 * t + s
                if dx > 14:
                    continue
                T[:, s, j + dy, t, j] = Wf[:, dy, dx][:, None]
    T = T.reshape(_NG, _G, 92, 8 * 32).transpose(0, 2, 1, 3)
    toep = np.ascontiguousarray(T).reshape(_NG, 92, _G * 8 * 32)
    return xpad, toep


def kernel(patches, kernels, kernel_size, patch_size, fft_size, _collect_results=None):
    """Full inputs in, full output out. Shards BN across 8 cores."""
    from concourse.bass_utils import run_bass_kernel_spmd

    patches = np.asarray(patches)
    kernels = np.asarray(kernels)
    assert patches.shape == (_BN, _C, _P, _P), patches.shape
    assert kernels.shape == (_BN, _C, _K, _K), kernels.shape

    if "nc" not in _nc_cache:
        _nc_cache["nc"] = _build_nc()
    nc = _nc_cache["nc"]

    bn_per_core = _BN // _N_CORES
    in_maps = []
    for core in range(_N_CORES):
        sl = slice(core * bn_per_core, (core + 1) * bn_per_core)
        pp = patches[sl].reshape(-1, _P, _P)
        kp = kernels[sl].reshape(-1, _K, _K)
        xpad, toep = _host_prep(pp, kp)
        in_maps.append({"xpad": xpad, "toep": toep})

    res = run_bass_kernel_spmd(nc, in_maps, core_ids=list(range(_N_CORES)))
    if _collect_results is not None:
        _collect_results.append(res)

    out = np.empty((_BN, _C, _P, _P), dtype=np.float32)
    for core in range(_N_CORES):
        sl = slice(core * bn_per_core, (core + 1) * bn_per_core)
        o = res.results[core]["out"].reshape(_NG, 128, _G, 128)
        out[sl] = o.transpose(0, 2, 1, 3).reshape(
            bn_per_core, _C, _P, _P).astype(np.float32)
    return out


# revision 6
# speedup vs baseline: 1.6090x; 1.3983x over previous
"""Trainium2 Bass kernel: depthwise (per-sample, per-channel) 15x15 'same'
true convolution of 1024x3 images of 128x128, data-parallel over 8 NeuronCores.

Formulation (per (bn,c) pair, P=128, K=15, pad=7):
    out[y,x] = sum_{dy,dx} Xp[y+dy, x+dx] * Wf[dy,dx],   Wf = flip(kernel),
    Xp = zero-padded image [142, 143].
Output rows are split into 4 blocks of 32 (j in 0..31). Each block runs on its
own 32-wide column strip of the PE array (tile_position=(0, 32b)) so the four
blocks' matmuls execute concurrently. Contraction (92) packs two dx taps:
segment s in {0,1} holds image rows 32b..32b+45 shifted s columns. Pass t
(t=0..7) covers dx = 2t+s via a moving-operand column offset of 2t;
the stationary Toeplitz slab T[46s+i, t, j] = Wf[i-j, 2t+s] accumulates all 8
passes into PSUM [32, 128] per block.

Data staging (per group of G=8 pairs): images are stored pair-interleaved in
DRAM ([row][pair][143]) so one DMA with 2288-byte runs fills segment 0 of the
x4 tile ([92, 4*G*143]); segment 1 (shift-by-one-column copy) is an
SBUF->SBUF DMA with one descriptor per partition. Toeplitz slabs and fp16
outputs are similarly group-batched. Sharding: pure data parallel over BN
(128 samples x 3 channels = 384 pairs per core).
"""
import sys

sys.path.insert(0, "/opt/trn_rl_repo")

import numpy as np

_N_CORES = 8
_BN, _C, _P, _K = 1024, 3, 128, 15
_PAIRS_PER_CORE = (_BN // _N_CORES) * _C  # 384
_G = 16                      # pairs per DMA group
_NG = _PAIRS_PER_CORE // _G  # 48
_XW = 143                    # padded image width (cols 0..142)
_XH = 142                    # padded image height
_ROWP = _G * _XW             # elems per padded row across a group (1144)
_GRP = _XH * _ROWP           # elems per group image block (162448)
_SLAB = 4 * _ROWP            # x4 tile free elems (4576) + 1 slack

_nc_cache = {}


def _build_nc(bufs: int = 2, psum_bufs: int = 4):
    import concourse.bacc as bacc
    import concourse.mybir as mybir
    from concourse import bass, tile

    FP16 = mybir.dt.float16
    FP32 = mybir.dt.float32

    nc = bacc.Bacc("TRN2", target_bir_lowering=False, debug=False)
    xpad_d = nc.dram_tensor("xpad", [_NG * _GRP + 64], FP16, kind="ExternalInput")
    toep_d = nc.dram_tensor("toep", [_NG, 92, _G * 8 * 32], FP16, kind="ExternalInput")
    out_d = nc.dram_tensor("out", [_NG, 128, _G * 128], FP16, kind="ExternalOutput")

    with tile.TileContext(nc) as tc:
        with (
            tc.tile_pool(name="x4", bufs=bufs) as x4_pool,
            tc.tile_pool(name="tt", bufs=bufs) as tt_pool,
            tc.tile_pool(name="ot", bufs=bufs) as ot_pool,
            tc.tile_pool(name="ps", bufs=psum_bufs, space="PSUM") as ps_pool,
        ):
            for grp in range(_NG):
                x4 = x4_pool.tile([92, _SLAB + 1], FP16, tag="x4")
                tt = tt_pool.tile([92, _G * 8 * 32], FP16, tag="tt")
                ot = ot_pool.tile([128, _G * 128], FP16, tag="ot")

                # seg0: partitions 0..45 <- rows 32b+i of each pair (interleaved)
                src0 = bass.AP(
                    tensor=xpad_d.tensor if hasattr(xpad_d, "tensor") else xpad_d,
                    offset=grp * _GRP,
                    ap=[[_ROWP, 46], [32 * _ROWP, 4], [_XW, _G], [1, _XW]],
                )
                nc.gpsimd.dma_start(out=x4[0:46, 0:_SLAB], in_=src0)
                # seg1: partitions 46..91 <- same, shifted one column
                nc.gpsimd.dma_start(out=x4[46:92, 0:_SLAB], in_=x4[0:46, 1:_SLAB + 1])
                # Toeplitz slabs
                nc.gpsimd.dma_start(out=tt[:], in_=toep_d[grp])

                for g in range(_G):
                    ps = ps_pool.tile([128, 128], FP32, tag="ps")
                    for t in range(8):
                        for b in range(4):
                            nc.tensor.matmul(
                                ps[32 * b:32 * b + 32, :],
                                tt[0:92, (g * 8 + t) * 32:(g * 8 + t) * 32 + 32],
                                x4[0:92, (b * _G + g) * _XW + 2 * t:
                                   (b * _G + g) * _XW + 2 * t + 128],
                                start=(t == 0), stop=(t == 7),
                                tile_position=(0, 32 * b),
                            )
                    nc.vector.tensor_copy(
                        ot[0:64, g * 128:(g + 1) * 128], ps[0:64, :])
                    nc.scalar.copy(
                        ot[64:128, g * 128:(g + 1) * 128], ps[64:128, :])

                nc.gpsimd.dma_start(out=out_d[grp], in_=ot[:])

    nc.compile()
    return nc


def _host_prep(patches_pairs: np.ndarray, kernels_pairs: np.ndarray):
    """[NP,128,128] f32, [NP,15,15] f32 -> (xpad flat fp16, toep fp16).

    xpad: [NG*142*G*143 + 64] with layout [grp][row 142][pair G][col 143],
    zero-padded images at rows/cols 7..134.
    toep: [NG, 92, G*8*32] with T[p][46s+i, t, j] = Wf[i-j, 2t+s]
    (0 <= i-j < 15, dx = 2t+s <= 14), layout [grp][i_stack][pair][t][j].
    """
    NP = patches_pairs.shape[0]
    assert NP == _PAIRS_PER_CORE
    Xp = np.zeros((_NG, _G, _XH, _XW), dtype=np.float16)
    Xp[:, :, 7:135, 7:135] = patches_pairs.reshape(_NG, _G, 128, 128)
    xpad = np.zeros(_NG * _GRP + 64, dtype=np.float16)
    xpad[:_NG * _GRP] = np.ascontiguousarray(
        Xp.transpose(0, 2, 1, 3)).reshape(-1)

    Wf = np.ascontiguousarray(
        kernels_pairs[:, ::-1, ::-1]).astype(np.float16)  # [NP, 15, 15]
    T = np.zeros((NP, 2, 46, 8, 32), dtype=np.float16)
    j = np.arange(32)
    for dy in range(15):
        for t in range(8):
            for s in range(2):
                dx = 2 * t + s
                if dx > 14:
                    continue
                T[:, s, j + dy, t, j] = Wf[:, dy, dx][:, None]
    T = T.reshape(_NG, _G, 92, 8 * 32).transpose(0, 2, 1, 3)
    toep = np.ascontiguousarray(T).reshape(_NG, 92, _G * 8 * 32)
    return xpad, toep


def kernel(patches, kernels, kernel_size, patch_size, fft_size, _collect_results=None):
    """Full inputs in, full output out. Shards BN across 8 cores."""
    from concourse.bass_utils import run_bass_kernel_spmd

    patches = np.asarray(patches)
    kernels = np.asarray(kernels)
    assert patches.shape == (_BN, _C, _P, _P), patches.shape
    assert kernels.shape == (_BN, _C, _K, _K), kernels.shape

    if "nc" not in _nc_cache:
        _nc_cache["nc"] = _build_nc()
    nc = _nc_cache["nc"]

    bn_per_core = _BN // _N_CORES
    in_maps = []
    for core in range(_N_CORES):
        sl = slice(core * bn_per_core, (core + 1) * bn_per_core)
        pp = patches[sl].reshape(-1, _P, _P)
        kp = kernels[sl].reshape(-1, _K, _K)
        xpad, toep = _host_prep(pp, kp)
        in_maps.append({"xpad": xpad, "toep": toep})

    res = run_bass_kernel_spmd(nc, in_maps, core_ids=list(range(_N_CORES)))
    if _collect_results is not None:
        _collect_results.append(res)

    out = np.empty((_BN, _C, _P, _P), dtype=np.float32)
    for core in range(_N_CORES):
        sl = slice(core * bn_per_core, (core + 1) * bn_per_core)
        o = res.results[core]["out"].reshape(_NG, 128, _G, 128)
        out[sl] = o.transpose(0, 2, 1, 3).reshape(
            bn_per_core, _C, _P, _P).astype(np.float32)
    return out
